# revision 27
# baseline (speedup 1.0000x reference)
"""Trainium2 Bass kernel for nn_BitwiseLinear.

Reference semantics (B=32768, IN=OUT=1024):
    out = in_scale * weight_scale * (sign(x) @ sign(weight * gate_mask).T + bias)
    gate_mask = (sign(gate)+1)/2; in_scale = mean|x| per row; weight_scale = mean|w| per out.

Identities used:
    sign(weight * gate_mask) == sign(weight) * (gate >= 0)   (gate==0 -> mask 0.5 -> sign(w))
    out = sum|x|_row * ws_eff * (signmm + bias),  ws_eff = sum|w|_row * 2^-20

Implementation: data-parallel over batch on 8 cores, weights replicated.
Per core: sign(x) tiles are PE-transposed (f32) into PSUM, Sign-activated into
fp8; binarized weights (+-1/0 in fp8, transposed to [i, o]) feed DoubleRow fp8
matmuls (K=256 per MM). Epilogue: psum * sum|x| (ACT scale) * ws_eff (TT).
"""

import numpy as np

import concourse.bacc as bacc
import concourse.mybir as mybir
import concourse.tile as tile
from concourse import masks
from concourse.bass_utils import run_bass_kernel_spmd

B, IN, OUT = 32768, 1024, 1024
NCORES = 8
BSH = B // NCORES            # 4096 rows per core
P = 128                      # partitions
NT = BSH // P                # 32 x-tiles per core
KC = IN // P                 # 8 contraction chunks of 128
NPAIR = KC // 2              # 4 DoubleRow K-pairs (256 each)
NCH = 512                    # matmul moving free-dim (one PSUM bank of f32)
F32 = mybir.dt.float32
BF16 = mybir.dt.bfloat16
FP8 = mybir.dt.float8e4
WS_SCALE = float(2.0 ** -20)  # 1/(1024*1024): folds both mean divisors

_CACHE: dict = {}


def _build(with_bias=True, with_gate=True):
    nc = bacc.Bacc("TRN2", target_bir_lowering=False, debug=False,
                   num_devices=NCORES)

    x_ext = nc.declare_dram_parameter("x", [BSH, IN], F32, isOutput=False)
    w_ext = nc.declare_dram_parameter("weight", [OUT, IN], F32, isOutput=False)
    g_ext = nc.declare_dram_parameter("gate", [OUT, IN], F32, isOutput=False)
    b_ext = nc.declare_dram_parameter("bias", [1, OUT], F32, isOutput=False)
    o_ext = nc.declare_dram_parameter("out", [BSH, OUT], F32, isOutput=True)

    x_ap = x_ext.ap()
    w_ap = w_ext.ap()
    g_ap = g_ext.ap()
    b_ap = b_ext.ap()
    o_ap = o_ext.ap()

    ACT = mybir.ActivationFunctionType
    ALU = mybir.AluOpType
    AX = mybir.AxisListType
    DR = mybir.MatmulPerfMode.DoubleRow

    with tile.TileContext(nc) as tc:
        with tc.tile_pool(name="const", bufs=1) as cp:
            ident_f32 = cp.tile([P, P], F32)
            masks.make_identity(nc, ident_f32[:])
            ident_bf = cp.tile([P, P], BF16)
            masks.make_identity(nc, ident_bf[:])
            ones_f8 = cp.tile([1, P], FP8)
            nc.gpsimd.memset(ones_f8[:], 1.0)
            ones_f32 = cp.tile([1, P], F32)
            nc.gpsimd.memset(ones_f32[:], 1.0)
            zbias = cp.tile([P, 1], F32)
            nc.gpsimd.memset(zbias[:], 0.0)

            # persistent prepped weights
            # pair j holds binarized wT chunks 2j (at [:, :OUT]) and 2j+1
            wtq = [cp.tile([P, 2 * OUT], FP8, tag=f"wtq{j}", name=f"wtq{j}") for j in range(NPAIR)]
            bias_f8 = cp.tile([1, OUT], FP8)      # raw bias (fp8) added pre-scale
            ws_bcast = cp.tile([P, OUT], F32)     # ws * 2^-20 broadcast over partitions

            # ---------------- weight prep (replicated on every core) --------
            with tc.tile_pool(name="wprep", bufs=2) as wp, \
                 tc.tile_pool(name="wkeep", bufs=1) as wk, \
                 tc.tile_pool(name="wpsum1", bufs=1, space="PSUM") as wps1, \
                 tc.tile_pool(name="wpsum", bufs=2, space="PSUM") as wps:
                # o-tile t: [128 o_t, 1024 i]
                w_bin = [wk.tile([P, IN], BF16, tag=f"wbin{t}", name=f"wbin{t}")
                         for t in range(KC)]
                ws_cols = wk.tile([P, KC], F32)   # per-o |w| row sums, tile t in col t
                bias_sb = wk.tile([1, OUT], F32)
                ws_row = wk.tile([1, OUT], F32)
                if with_bias:
                    nc.sync.dma_start(bias_sb[:], b_ap[:, :])
                    nc.vector.tensor_copy(bias_f8[:], bias_sb[:])

                for t in range(KC):
                    wt = wp.tile([P, IN], F32)
                    nc.gpsimd.dma_start(wt[:], w_ap[t * P:(t + 1) * P, :])
                    nc.vector.tensor_reduce(ws_cols[:, t:t + 1], wt[:], axis=AX.X,
                                            op=ALU.add, apply_absolute_value=True)
                    if with_gate:
                        gt = wp.tile([P, IN], F32)
                        nc.gpsimd.dma_start(gt[:], g_ap[t * P:(t + 1) * P, :])
                        sgn = wp.tile([P, IN], BF16)
                        nc.scalar.activation(sgn[:], wt[:], ACT.Sign, bias=zbias[:])
                        msk = wp.tile([P, IN], BF16)
                        nc.vector.tensor_scalar(msk[:], gt[:], 0.0, None,
                                                op0=ALU.is_ge)
                        nc.vector.tensor_tensor(w_bin[t][:], sgn[:], msk[:],
                                                op=ALU.mult)
                    else:
                        nc.scalar.activation(w_bin[t][:], wt[:], ACT.Sign,
                                             bias=zbias[:])

                # wtq pair j, half h = transpose(w_bin)[chunk 2j+h rows, all o]
                for c in range(KC):
                    ps_wt = wps.tile([P, OUT], BF16, tag="ps_wt")
                    for t in range(KC):
                        nc.tensor.transpose(
                            ps_wt[:, t * P:(t + 1) * P],
                            w_bin[t][:, c * P:(c + 1) * P],
                            ident_bf[:])
                    dst = wtq[c // 2][:, (c % 2) * OUT:((c % 2) + 1) * OUT]
                    if c % 2:
                        nc.scalar.copy(dst, ps_wt[:])
                    else:
                        nc.vector.tensor_copy(dst, ps_wt[:])

                # ws_row[0, o] = sum_i |w[o, i]| * 2^-20, via 8 tiny PE transposes
                ps_row = wps1.tile([1, OUT], F32)
                for t in range(KC):
                    nc.tensor.transpose(ps_row[0:1, t * P:(t + 1) * P],
                                        ws_cols[:, t:t + 1], ident_f32[:])
                nc.scalar.activation(ws_row[:], ps_row[:], ACT.Copy, scale=WS_SCALE)

                # broadcast ws_row across partitions with a K=1 matmul
                ps_bc = wps1.tile([P, OUT], F32, tag="ps_row")
                for n in range(OUT // NCH):
                    nc.tensor.matmul(ps_bc[:, n * NCH:(n + 1) * NCH], ones_f32[:],
                                     ws_row[:, n * NCH:(n + 1) * NCH])
                nc.vector.tensor_copy(ws_bcast[:], ps_bc[:])

            # ---------------- main loop over x tiles ----------------
            with tc.tile_pool(name="xin", bufs=8) as xin_pool, \
                 tc.tile_pool(name="xbt", bufs=8) as xbt_pool, \
                 tc.tile_pool(name="osb", bufs=3) as osb_pool, \
                 tc.tile_pool(name="sc", bufs=6) as sc_pool, \
                 tc.tile_pool(name="pst", bufs=4, space="PSUM") as pst_pool, \
                 tc.tile_pool(name="pso", bufs=4, space="PSUM") as pso_pool:

                xts = [None] * NT
                xbts = [None] * NT
                is_raws = [None] * NT
                out_sbs = [None] * NT

                def stage_front(it):
                    """DMA in + PE transpose + fp8 sign (2 halves)."""
                    xt = xin_pool.tile([P, IN], F32, tag="xt", name=f"xt{it}")
                    nc.sync.dma_start(xt[:], x_ap[it * P:(it + 1) * P, :])
                    xts[it] = xt
                    xbT = []
                    for h in range(2):
                        ps_t = pst_pool.tile([P, NCH], F32, tag="ps_t")
                        for ci in range(KC // 2):
                            c = h * (KC // 2) + ci
                            nc.tensor.transpose(ps_t[:, ci * P:(ci + 1) * P],
                                                xt[:, c * P:(c + 1) * P],
                                                ident_f32[:])
                        xbh = xbt_pool.tile([P, NCH], FP8, tag="xbT",
                                            name=f"xbT{h}")
                        nc.scalar.activation(xbh[:], ps_t[:], ACT.Sign, bias=zbias[:])
                        xbT.append(xbh)
                    xbts[it] = xbT

                def stage_back(it):
                    """Deferred final scale + store (one iter later)."""
                    out2 = osb_pool.tile([P, OUT], F32, tag="out2")
                    nc.vector.tensor_scalar(out2[:], out_sbs[it][:], is_raws[it][:],
                                            None, op0=ALU.mult)
                    nc.gpsimd.dma_start(o_ap[it * P:(it + 1) * P, :], out2[:])

                for pre in range(3):
                    stage_front(pre)
                for it in range(NT):
                    xbT = xbts[it]
                    out_sb = osb_pool.tile([P, OUT], F32, tag="out_sb")
                    ps_os = []
                    for n in range(OUT // NCH):
                        ps_os.append(pso_pool.tile([P, NCH], F32, tag="ps_o",
                                                   name=f"ps_o{n}"))
                    for j in range(NPAIR):
                        h, jj = divmod(j, NPAIR // 2)
                        xp = xbT[h][:, jj * 2 * P:(jj + 1) * 2 * P].rearrange(
                            "p (two m) -> p two m", two=2)
                        wq = wtq[j][:].rearrange("p (two o) -> p two o", two=2)
                        for n in range(OUT // NCH):
                            nc.tensor.matmul(
                                ps_os[n][:],
                                xp,
                                wq[:, :, n * NCH:(n + 1) * NCH],
                                start=(j == 0),
                                stop=(not with_bias and j == NPAIR - 1),
                                perf_mode=DR)
                    for n in range(OUT // NCH):
                        if with_bias:
                            nc.tensor.matmul(ps_os[n][:], ones_f8[:],
                                             bias_f8[:, n * NCH:(n + 1) * NCH],
                                             start=False, stop=True)
                        nc.vector.tensor_tensor(out_sb[:, n * NCH:(n + 1) * NCH],
                                                ps_os[n][:],
                                                ws_bcast[:, n * NCH:(n + 1) * NCH],
                                                op=ALU.mult)
                    out_sbs[it] = out_sb

                    if it >= 1:
                        stage_back(it - 1)
                    if it + 3 < NT:
                        stage_front(it + 3)

                    # row abs-sums on ACT (Abs writes a throwaway tile, the
                    # accumulator gives sum|x| per row); needed 2 iters later
                    def emit_reduce(which):
                        is_raw = sc_pool.tile([P, 1], F32, tag="is_raw",
                                              name=f"is_raw{which}")
                        if which % 5 == 4:
                            nc.vector.tensor_reduce(
                                is_raw[:], xts[which][:], axis=AX.X,
                                op=ALU.add, apply_absolute_value=True)
                        else:
                            scr = sc_pool.tile([P, IN], BF16, tag="abs_scr",
                                               name="abs_scr")
                            nc.scalar.activation(scr[:], xts[which][:], ACT.Abs,
                                                 bias=zbias[:],
                                                 accum_out=is_raw[:])
                        is_raws[which] = is_raw

                    if it == 0:
                        for pre in range(3):
                            emit_reduce(pre)
                    if it + 3 < NT:
                        emit_reduce(it + 3)
                stage_back(NT - 1)

    nc.compile()
    return nc


def _get_nc(with_bias, with_gate):
    key = f"nc{int(with_bias)}{int(with_gate)}"
    if key not in _CACHE:
        _CACHE[key] = _build(with_bias, with_gate)
    return _CACHE[key]


def run(x, weight, gate, bias, trace=False):
    # gate >= 0 everywhere makes the gate mask exactly 1 (sign(g)+1)/2 with
    # g==0 -> 0.5, and sign(w*0.5) == sign(w)); skip it entirely then.
    nc = _get_nc(bool(np.any(np.asarray(bias))),
                 not bool(np.all(np.asarray(gate) >= 0.0)))
    x = np.ascontiguousarray(np.asarray(x, dtype=np.float32))
    weight = np.ascontiguousarray(np.asarray(weight, dtype=np.float32))
    gate = np.ascontiguousarray(np.asarray(gate, dtype=np.float32))
    bias = np.ascontiguousarray(np.asarray(bias, dtype=np.float32)).reshape(1, OUT)
    in_maps = [
        {"x": x[i * BSH:(i + 1) * BSH], "weight": weight, "gate": gate, "bias": bias}
        for i in range(NCORES)
    ]
    res = run_bass_kernel_spmd(nc, in_maps, core_ids=list(range(NCORES)), trace=trace)
    out = np.concatenate([res.results[i]["out"] for i in range(NCORES)], axis=0)
    return out, res


def kernel(x, weight, gate, bias):
    out, _ = run(x, weight, gate, bias, trace=False)
    return out


# revision 29
# speedup vs baseline: 1.1333x; 1.1333x over previous
"""Trainium2 Bass kernel for nn_BitwiseLinear.

Reference semantics (B=32768, IN=OUT=1024):
    out = in_scale * weight_scale * (sign(x) @ sign(weight * gate_mask).T + bias)
    gate_mask = (sign(gate)+1)/2; in_scale = mean|x| per row; weight_scale = mean|w| per out.

Identities used:
    sign(weight * gate_mask) == sign(weight) * (gate >= 0)   (gate==0 -> mask 0.5 -> sign(w))
    out = sum|x|_row * ws_eff * (signmm + bias),  ws_eff = sum|w|_row * 2^-20

Implementation: data-parallel over batch on 8 cores, weights replicated.
Per core: sign(x) tiles are PE-transposed (f32) into PSUM, Sign-activated into
fp8; binarized weights (+-1/0 in fp8, transposed to [i, o]) feed DoubleRow fp8
matmuls (K=256 per MM). Epilogue: psum * sum|x| (ACT scale) * ws_eff (TT).
"""

import numpy as np

import concourse.bacc as bacc
import concourse.mybir as mybir
import concourse.tile as tile
from concourse import masks
from concourse.bass_utils import run_bass_kernel_spmd

B, IN, OUT = 32768, 1024, 1024
NCORES = 8
BSH = B // NCORES            # 4096 rows per core
P = 128                      # partitions
NT = BSH // P                # 32 x-tiles per core
KC = IN // P                 # 8 contraction chunks of 128
NPAIR = KC // 2              # 4 DoubleRow K-pairs (256 each)
NCH = 512                    # matmul moving free-dim (one PSUM bank of f32)
F32 = mybir.dt.float32
BF16 = mybir.dt.bfloat16
FP8 = mybir.dt.float8e4
WS_SCALE = float(2.0 ** -20)  # 1/(1024*1024): folds both mean divisors

_CACHE: dict = {}


def _build(with_bias=True, with_gate=True):
    nc = bacc.Bacc("TRN2", target_bir_lowering=False, debug=False,
                   num_devices=NCORES)

    x_ext = nc.declare_dram_parameter("x", [BSH, IN], F32, isOutput=False)
    w_ext = nc.declare_dram_parameter("weight", [OUT, IN], F32, isOutput=False)
    g_ext = nc.declare_dram_parameter("gate", [OUT, IN], F32, isOutput=False)
    b_ext = nc.declare_dram_parameter("bias", [1, OUT], F32, isOutput=False)
    o_ext = nc.declare_dram_parameter("out", [BSH, OUT], F32, isOutput=True)

    x_ap = x_ext.ap()
    w_ap = w_ext.ap()
    g_ap = g_ext.ap()
    b_ap = b_ext.ap()
    o_ap = o_ext.ap()

    ACT = mybir.ActivationFunctionType
    ALU = mybir.AluOpType
    AX = mybir.AxisListType
    DR = mybir.MatmulPerfMode.DoubleRow

    with tile.TileContext(nc) as tc:
        with tc.tile_pool(name="const", bufs=1) as cp:
            ident_f32 = cp.tile([P, P], F32)
            masks.make_identity(nc, ident_f32[:])
            ident_bf = cp.tile([P, P], BF16)
            masks.make_identity(nc, ident_bf[:])
            ones_f8 = cp.tile([1, P], FP8)
            nc.gpsimd.memset(ones_f8[:], 1.0)
            ones_f32 = cp.tile([1, P], F32)
            nc.gpsimd.memset(ones_f32[:], 1.0)
            zbias = cp.tile([P, 1], F32)
            nc.gpsimd.memset(zbias[:], 0.0)

            # persistent prepped weights
            # pair j holds binarized wT chunks 2j (at [:, :OUT]) and 2j+1
            wtq = [cp.tile([P, 2 * OUT], FP8, tag=f"wtq{j}", name=f"wtq{j}") for j in range(NPAIR)]
            bias_f8 = cp.tile([1, OUT], FP8)      # raw bias (fp8) added pre-scale
            ws_bcast = cp.tile([P, OUT], F32)     # ws * 2^-20 broadcast over partitions

            # ---------------- weight prep (replicated on every core) --------
            with tc.tile_pool(name="wprep", bufs=2) as wp, \
                 tc.tile_pool(name="wkeep", bufs=1) as wk, \
                 tc.tile_pool(name="wpsum1", bufs=1, space="PSUM") as wps1, \
                 tc.tile_pool(name="wpsum", bufs=2, space="PSUM") as wps:
                # o-tile t: [128 o_t, 1024 i]
                w_bin = [wk.tile([P, IN], BF16, tag=f"wbin{t}", name=f"wbin{t}")
                         for t in range(KC)]
                ws_cols = wk.tile([P, KC], F32)   # per-o |w| row sums, tile t in col t
                bias_sb = wk.tile([1, OUT], F32)
                ws_row = wk.tile([1, OUT], F32)
                if with_bias:
                    nc.sync.dma_start(bias_sb[:], b_ap[:, :])
                    nc.vector.tensor_copy(bias_f8[:], bias_sb[:])

                wt4 = [wp.tile([P, 4 * IN], F32, tag=f"wt4_{q}",
                               name=f"wt4_{q}") for q in range(2)]
                gt4 = []
                for q in range(2):
                    nc.gpsimd.dma_start(
                        wt4[q][:].rearrange("p (t i) -> p t i", t=4),
                        w_ap[q * 4 * P:(q + 1) * 4 * P, :].rearrange(
                            "(t p) i -> p t i", p=P))
                    if with_gate:
                        g4 = wp.tile([P, 4 * IN], F32, tag=f"gt4_{q}",
                                     name=f"gt4_{q}")
                        nc.gpsimd.dma_start(
                            g4[:].rearrange("p (t i) -> p t i", t=4),
                            g_ap[q * 4 * P:(q + 1) * 4 * P, :].rearrange(
                                "(t p) i -> p t i", p=P))
                        gt4.append(g4)
                for t in range(KC):
                    wt = wt4[t // 4][:, (t % 4) * IN:((t % 4) + 1) * IN]
                    nc.vector.tensor_reduce(ws_cols[:, t:t + 1], wt, axis=AX.X,
                                            op=ALU.add, apply_absolute_value=True)
                    if with_gate:
                        gt = gt4[t // 4][:, (t % 4) * IN:((t % 4) + 1) * IN]
                        sgn = wp.tile([P, IN], BF16)
                        nc.scalar.activation(sgn[:], wt, ACT.Sign, bias=zbias[:])
                        msk = wp.tile([P, IN], BF16)
                        nc.vector.tensor_scalar(msk[:], gt, 0.0, None,
                                                op0=ALU.is_ge)
                        nc.vector.tensor_tensor(w_bin[t][:], sgn[:], msk[:],
                                                op=ALU.mult)
                    else:
                        nc.scalar.activation(w_bin[t][:], wt, ACT.Sign,
                                             bias=zbias[:])

                # wtq pair j, half h = transpose(w_bin)[chunk 2j+h rows, all o]
                for c in range(KC):
                    ps_wt = wps.tile([P, OUT], BF16, tag="ps_wt")
                    for t in range(KC):
                        nc.tensor.transpose(
                            ps_wt[:, t * P:(t + 1) * P],
                            w_bin[t][:, c * P:(c + 1) * P],
                            ident_bf[:])
                    dst = wtq[c // 2][:, (c % 2) * OUT:((c % 2) + 1) * OUT]
                    if c % 2:
                        nc.scalar.copy(dst, ps_wt[:])
                    else:
                        nc.vector.tensor_copy(dst, ps_wt[:])

                # ws_row[0, o] = sum_i |w[o, i]| * 2^-20, via 8 tiny PE transposes
                ps_row = wps1.tile([1, OUT], F32)
                for t in range(KC):
                    nc.tensor.transpose(ps_row[0:1, t * P:(t + 1) * P],
                                        ws_cols[:, t:t + 1], ident_f32[:])
                nc.scalar.activation(ws_row[:], ps_row[:], ACT.Copy, scale=WS_SCALE)

                # broadcast ws_row across partitions with a K=1 matmul
                ps_bc = wps1.tile([P, OUT], F32, tag="ps_row")
                for n in range(OUT // NCH):
                    nc.tensor.matmul(ps_bc[:, n * NCH:(n + 1) * NCH], ones_f32[:],
                                     ws_row[:, n * NCH:(n + 1) * NCH])
                nc.vector.tensor_copy(ws_bcast[:], ps_bc[:])

            # ---------------- main loop over x tiles ----------------
            # DMA granularity is a PAIR of 128-row tiles (1MB transfers);
            # compute granularity stays one tile.
            NT2 = NT // 2
            with tc.tile_pool(name="xin", bufs=4) as xin_pool, \
                 tc.tile_pool(name="xbt", bufs=8) as xbt_pool, \
                 tc.tile_pool(name="osb", bufs=3) as osb_pool, \
                 tc.tile_pool(name="opair", bufs=3) as opair_pool, \
                 tc.tile_pool(name="sc", bufs=6) as sc_pool, \
                 tc.tile_pool(name="pst", bufs=4, space="PSUM") as pst_pool, \
                 tc.tile_pool(name="pso", bufs=4, space="PSUM") as pso_pool:

                xts = [None] * NT
                xbts = [None] * NT
                is_raws = [None] * NT
                out_sbs = [None] * NT
                out_pairs = [None] * NT2

                def stage_front_pair(m):
                    """1MB DMA for tiles 2m, 2m+1 + their transposes + signs."""
                    xt2 = xin_pool.tile([P, 2 * IN], F32, tag="xt2",
                                        name=f"xt2_{m}")
                    nc.sync.dma_start(
                        xt2[:].rearrange("p (t i) -> p t i", t=2),
                        x_ap[m * 2 * P:(m + 1) * 2 * P, :].rearrange(
                            "(t p) i -> p t i", p=P))
                    for t in range(2):
                        it = 2 * m + t
                        xt = xt2[:, t * IN:(t + 1) * IN]
                        xts[it] = xt
                        xbT = []
                        for h in range(2):
                            ps_t = pst_pool.tile([P, NCH], F32, tag="ps_t")
                            for ci in range(KC // 2):
                                c = h * (KC // 2) + ci
                                nc.tensor.transpose(ps_t[:, ci * P:(ci + 1) * P],
                                                    xt[:, c * P:(c + 1) * P],
                                                    ident_f32[:])
                            xbh = xbt_pool.tile([P, NCH], FP8, tag="xbT",
                                                name=f"xbT{h}")
                            nc.scalar.activation(xbh[:], ps_t[:], ACT.Sign,
                                                 bias=zbias[:])
                            xbT.append(xbh)
                        xbts[it] = xbT

                def stage_back(it):
                    """Deferred final scale into the pair buffer; 1MB DMA out
                    when the second half of the pair lands."""
                    m, t = divmod(it, 2)
                    if t == 0:
                        out_pairs[m] = opair_pool.tile([P, 2 * OUT], F32,
                                                       tag="opair",
                                                       name=f"opair{m}")
                    dst = out_pairs[m][:, t * OUT:(t + 1) * OUT]
                    nc.vector.tensor_scalar(dst, out_sbs[it][:], is_raws[it][:],
                                            None, op0=ALU.mult)
                    if t == 1:
                        nc.gpsimd.dma_start(
                            o_ap[m * 2 * P:(m + 1) * 2 * P, :].rearrange(
                                "(u p) o -> p u o", p=P),
                            out_pairs[m][:].rearrange("p (u o) -> p u o", u=2))

                def emit_reduce(which):
                    if which >= NT:
                        return
                    is_raw = sc_pool.tile([P, 1], F32, tag="is_raw",
                                          name=f"is_raw{which}")
                    if which % 5 == 4:
                        nc.vector.tensor_reduce(
                            is_raw[:], xts[which], axis=AX.X,
                            op=ALU.add, apply_absolute_value=True)
                    else:
                        scr = sc_pool.tile([P, IN], BF16, tag="abs_scr",
                                           name="abs_scr")
                        nc.scalar.activation(scr[:], xts[which], ACT.Abs,
                                             bias=zbias[:], accum_out=is_raw[:])
                    is_raws[which] = is_raw

                stage_front_pair(0)
                stage_front_pair(1)
                for it in range(NT):
                    xbT = xbts[it]
                    out_sb = osb_pool.tile([P, OUT], F32, tag="out_sb")
                    ps_os = []
                    for n in range(OUT // NCH):
                        ps_os.append(pso_pool.tile([P, NCH], F32, tag="ps_o",
                                                   name=f"ps_o{n}"))
                    for j in range(NPAIR):
                        h, jj = divmod(j, NPAIR // 2)
                        xp = xbT[h][:, jj * 2 * P:(jj + 1) * 2 * P].rearrange(
                            "p (two m) -> p two m", two=2)
                        wq = wtq[j][:].rearrange("p (two o) -> p two o", two=2)
                        for n in range(OUT // NCH):
                            nc.tensor.matmul(
                                ps_os[n][:],
                                xp,
                                wq[:, :, n * NCH:(n + 1) * NCH],
                                start=(j == 0),
                                stop=(not with_bias and j == NPAIR - 1),
                                perf_mode=DR)
                    for n in range(OUT // NCH):
                        if with_bias:
                            nc.tensor.matmul(ps_os[n][:], ones_f8[:],
                                             bias_f8[:, n * NCH:(n + 1) * NCH],
                                             start=False, stop=True)
                        nc.vector.tensor_tensor(out_sb[:, n * NCH:(n + 1) * NCH],
                                                ps_os[n][:],
                                                ws_bcast[:, n * NCH:(n + 1) * NCH],
                                                op=ALU.mult)
                    out_sbs[it] = out_sb

                    if it >= 1:
                        stage_back(it - 1)
                    if it % 2 == 0 and it // 2 + 2 < NT2:
                        stage_front_pair(it // 2 + 2)
                    if it == 0:
                        for pre in range(4):
                            emit_reduce(pre)
                    if it % 2 == 0:
                        emit_reduce(it + 4)
                        emit_reduce(it + 5)
                stage_back(NT - 1)

    nc.compile()
    return nc


def _get_nc(with_bias, with_gate):
    key = f"nc{int(with_bias)}{int(with_gate)}"
    if key not in _CACHE:
        _CACHE[key] = _build(with_bias, with_gate)
    return _CACHE[key]


def run(x, weight, gate, bias, trace=False):
    # gate >= 0 everywhere makes the gate mask exactly 1 (sign(g)+1)/2 with
    # g==0 -> 0.5, and sign(w*0.5) == sign(w)); skip it entirely then.
    nc = _get_nc(bool(np.any(np.asarray(bias))),
                 not bool(np.all(np.asarray(gate) >= 0.0)))
    x = np.ascontiguousarray(np.asarray(x, dtype=np.float32))
    weight = np.ascontiguousarray(np.asarray(weight, dtype=np.float32))
    gate = np.ascontiguousarray(np.asarray(gate, dtype=np.float32))
    bias = np.ascontiguousarray(np.asarray(bias, dtype=np.float32)).reshape(1, OUT)
    in_maps = [
        {"x": x[i * BSH:(i + 1) * BSH], "weight": weight, "gate": gate, "bias": bias}
        for i in range(NCORES)
    ]
    res = run_bass_kernel_spmd(nc, in_maps, core_ids=list(range(NCORES)), trace=trace)
    out = np.concatenate([res.results[i]["out"] for i in range(NCORES)], axis=0)
    return out, res


def kernel(x, weight, gate, bias):
    out, _ = run(x, weight, gate, bias, trace=False)
    return out
